# revision 1
# baseline (speedup 1.0000x reference)
"""Trainium2 Bass kernel for a dense pre-norm transformer block.

Problem: x[8, 1024, 768]; per-batch-element transformer block
  (LN1 -> qkv -> 12-head attention -> proj residual -> LN2 -> MLP(gelu) residual).

Strategy (v2):
  - Pure data-parallel: 8 NeuronCores, one batch element each. No collectives.
  - Activations channel-major ("T layout", [C, tokens]); host transposes.
  - Residual stream kept in f32r so LN-stats matmuls read it directly (no
    cast copies); evictions write f32r.
  - Weights stored fp8e4 scaled x16 (stationary operands only; wv stays bf16
    as it is a moving operand). LN gains folded into the following weights,
    LN biases folded into the following biases host-side, so the on-chip LN
    apply is just (x - mu) * rstd.
  - rstd = exp(-0.5*ln(var+eps)): keeps ScalarE in the natural_log_exp table
    set through LN1/attention/LN2; only gelu forces a table switch (2/rep).
  - Softmax: transposed scores; denominators via a ones-column in V staged to
    a [12, N] tile via small DMAs; one reciprocal_approx_fast over all 12
    rows; broadcast across partitions with a K=2 select matmul into PSUM
    during proj (no DRAM round trip).
  - No tiny-descriptor DMAs (the ones_v broadcast DMA of the old version
    poisoned the DMA engines for ~120us each rep).
  - Weight DMAs: few, large, on the idle sync queue in need-order; MLP
    streams double-buffered 6 deep so the next rep's qkv weights are in
    flight by the time fc2 drains.
"""

import ml_dtypes
import numpy as np

import concourse.bacc as bacc
import concourse.bass as bass
import concourse.mybir as mybir
from concourse import tile
from concourse.bass_utils import run_bass_kernel_spmd

AF = mybir.ActivationFunctionType
ALU = mybir.AluOpType
f32 = mybir.dt.float32
f32r = mybir.dt.float32r
bf16 = mybir.dt.bfloat16
fp8 = mybir.dt.float8e4

P = 128
DIM = 768
CT = DIM // P            # 6 channel tiles
N = 1024                 # tokens
NT = N // P              # 8 token tiles
NH = 12                  # heads
DH = 64                  # head dim
HID = 3072
HT = HID // P            # 24 hidden tiles
B = 8
EPS = 1e-5
SCALE = DH ** -0.5
WS = 16.0                # fp8 weight pre-scale
IWS = 1.0 / WS


def _t6(dram_2d):
    return dram_2d.rearrange("(a p) m -> p a m", p=P)


def build_nc(reps=1):
    nc = bacc.Bacc("TRN2", target_bir_lowering=False, debug=False)

    xT = nc.dram_tensor("xT", [DIM, N], f32, kind="ExternalInput")
    wqk = nc.dram_tensor("wqk", [12, P, CT * P], fp8, kind="ExternalInput")
    wv = nc.dram_tensor("wv", [P, CT, DIM], bf16, kind="ExternalInput")
    wproj = nc.dram_tensor("wproj", [P, CT, DIM], fp8, kind="ExternalInput")
    wfc1 = nc.dram_tensor("wfc1", [HT, P, CT * P], fp8, kind="ExternalInput")
    wfc2 = nc.dram_tensor("wfc2", [HT, 2, P, 3 * P], fp8, kind="ExternalInput")
    # consts cols: 0:12 bqk*16 | 12:36 bfc1 | 36:42 bproj | 42:48 bfc2
    consts = nc.dram_tensor("consts", [P, 48], f32, kind="ExternalInput")
    vb = nc.dram_tensor("vb", [DIM], f32, kind="ExternalInput")
    selS = nc.dram_tensor("selS", [12, CT * P], f32, kind="ExternalInput")
    outT = nc.dram_tensor("outT", [DIM, N], f32, kind="ExternalOutput")

    args = locals()
    with tile.TileContext(nc) as tc:
        _body(nc, tc, args, reps)
    nc.compile()
    return nc


def _body(nc, tc, t, reps=1):
    xT, outT = t["xT"], t["outT"]
    wqk, wv, wproj, wfc1, wfc2 = t["wqk"], t["wv"], t["wproj"], t["wfc1"], t["wfc2"]

    with (
        tc.tile_pool(name="const", bufs=1) as const,
        tc.tile_pool(name="resid", bufs=1) as resid,
        tc.tile_pool(name="hpool", bufs=1) as hpool,
    ):
        # ---- residual stream (channel-major, f32r) ----
        xsb = resid.tile([P, CT, N], f32r)
        for ct in range(CT):
            nc.sync.dma_start(xsb[:, ct, :],
                              xT[ct * P:(ct + 1) * P, :].bitcast(f32r))

        # ---- constants ----
        ones_f = const.tile([P, P], f32)
        nc.vector.memset(ones_f[:], 1.0 / DIM)
        ones_r = const.tile([P, P], f32r)
        nc.scalar.copy(ones_r[:], ones_f[:])
        eps_t = const.tile([P, 1], f32)
        nc.vector.memset(eps_t[:], EPS)
        cpack = const.tile([P, 48], f32)
        nc.sync.dma_start(cpack[:], t["consts"][:])
        bqk_sb = cpack[:, 0:12]
        bfc1_sb = cpack[:, 12:36]
        bproj_sb = cpack[:, 36:42]
        bfc2_sb = cpack[:, 42:48]
        selS_sb = const.tile([12, CT * P], f32r)
        nc.sync.dma_start(selS_sb[:], t["selS"][:].bitcast(f32r))
        dum = const.tile([P, 1], f32)
        def prefetch(af):
            nc.scalar.activation(dum[:], eps_t[:], af)
        prefetch(AF.Sqrt)
        vb_sb = const.tile([P, DIM], f32)
        vb_ap = t["vb"][:]
        nc.gpsimd.dma_start(
            vb_sb[:], bass.AP(tensor=vb_ap.tensor, offset=vb_ap.offset,
                              ap=[[0, P], [1, DIM]]))

        def layer_norm_T(dst):
            """dst := (xsb - mu) * rstd  (gains/biases folded into weights)."""
            with (
                tc.tile_pool(name="ln_tmp", bufs=1) as tmp,
                tc.tile_pool(name="ln_ps", bufs=1, space="PSUM") as lps,
            ):
                mu_ps = lps.tile([P, N], f32, tag="lnps", bufs=2,
                                 name="mu_ps")
                e2_ps = lps.tile([P, N], f32, tag="lnps", bufs=2,
                                 name="e2_ps")
                for ct in range(CT):
                    sq = tmp.tile([P, N], f32r, tag="sq", bufs=2, name="sq")
                    nc.scalar.square(sq[:], xsb[:, ct, :].bitcast(f32))
                    for h in range(2):
                        sl = bass.ts(h, 512)
                        nc.tensor.matmul(
                            mu_ps[:, sl], ones_r[:], xsb[:, ct, sl],
                            start=(ct == 0), stop=(ct == CT - 1))
                        nc.tensor.matmul(
                            e2_ps[:, sl], ones_r[:], sq[:, sl],
                            start=(ct == 0), stop=(ct == CT - 1))
                mu2 = tmp.tile([P, N], f32, tag="lt", bufs=2, name="mu2")
                nc.scalar.square(mu2[:], mu_ps[:])
                mu_sb = tmp.tile([P, N], f32, tag="mu", name="mu_sb")
                nc.vector.tensor_copy(mu_sb[:], mu_ps[:])
                var = tmp.tile([P, N], f32, tag="lt", bufs=2, name="var")
                nc.vector.tensor_sub(var[:], e2_ps[:], mu2[:])
                sd = tmp.tile([P, N], f32, tag="lt", bufs=2, name="sd")
                nc.scalar.activation(sd[:], var[:], AF.Sqrt, bias=eps_t[:],
                                     scale=1.0)
                rstd = tmp.tile([P, N], f32, tag="rstd", name="rstd")
                nc.vector.reciprocal_approx_fast(rstd[:], sd[:])
                for ct in range(CT):
                    t1 = tmp.tile([P, N], bf16, tag="t1", bufs=3, name="t1")
                    eng = nc.gpsimd if ct % 2 else nc.vector
                    eng.tensor_sub(t1[:], xsb[:, ct, :].bitcast(f32),
                                   mu_sb[:])
                    nc.vector.tensor_mul(dst[:, ct, :], t1[:], rstd[:])

        for _rep in range(reps):
            last = _rep == reps - 1
            with tc.tile_pool(name="repw", bufs=1) as qw:
                # ---- weight loads, need-order, all on the sync queue ----
                wqk_sb = qw.tile([P, 12, CT * P], fp8, name="wqk_sb")
                for c in range(3):
                    nc.sync.dma_start(
                        wqk_sb[:, 4 * c:4 * c + 4, :],
                        wqk[4 * c:4 * c + 4, :, :].rearrange("t p m -> p t m"))
                wv_sb = qw.tile([P, CT, DIM], bf16, name="wv_sb")
                nc.sync.dma_start(wv_sb[:], wv[:])
                wp_sb = qw.tile([P, CT, DIM], fp8, name="wp_sb")
                nc.sync.dma_start(wp_sb[:], wproj[:])
                w1tiles = []
                for c in range(12):
                    w1t = qw.tile([P, 2, CT * P], fp8, tag="w1t", bufs=6,
                                  name="w1t")
                    nc.sync.dma_start(
                        w1t[:],
                        wfc1[2 * c:2 * c + 2, :, :].rearrange(
                            "t p m -> p t m"))
                    w1tiles.append(w1t)
                w2tiles = []
                for c in range(HT // 2):
                    w2t = qw.tile([P, 2, 2, 3 * P], fp8, tag="w2t",
                                  bufs=HT // 2, name="w2t")
                    nc.sync.dma_start(
                        w2t[:],
                        wfc2[2 * c:2 * c + 2, :, :, :].rearrange(
                            "t g p m -> p t g m"))
                    w2tiles.append(w2t)

                # ======== LN1 ========
                h1 = hpool.tile([P, CT, N], bf16, tag="h", name="h1")
                layer_norm_T(h1)
                prefetch(AF.Exp)

                with tc.tile_pool(name="attn", bufs=1) as attn:
                    vsb = attn.tile([P, NT, NH, DH + 1], bf16, name="vsb")
                    osb = attn.tile([P, CT, N], bf16, name="osb")
                    dtmp = attn.tile([12, N], f32, name="dtmp")
                    nc.gpsimd.memset(
                        vsb[:, :, :, DH].rearrange("p a b -> p (a b)"), 1.0)

                    with (
                        tc.tile_pool(name="att_sb", bufs=1) as asb,
                        tc.tile_pool(name="att_ps", bufs=1,
                                     space="PSUM") as aps,
                    ):
                        def v_tile(it):
                            vps = aps.tile([P, N], f32, tag="sc", bufs=2,
                                           name="vps")
                            for c0, cn in ((0, 512), (512, 256)):
                                for kt in range(CT):
                                    nc.tensor.matmul(
                                        vps[:, c0:c0 + cn],
                                        h1[:, kt, it * P:(it + 1) * P],
                                        wv_sb[:, kt, c0:c0 + cn],
                                        start=(kt == 0), stop=(kt == CT - 1))
                            nc.vector.scalar_tensor_tensor(
                                out=vsb[:, it, :, 0:DH],
                                in0=vps[:, 0:DIM].rearrange(
                                    "p (h d) -> p h d", d=DH),
                                scalar=1.0, op0=ALU.mult,
                                in1=vb_sb[:].rearrange("p (h d) -> p h d",
                                                       d=DH),
                                op1=ALU.add)

                        def qk_prod(tp):
                            # q tile: both heads stacked on partitions
                            qkps = aps.tile([P, N], f32, tag="sc",
                                            bufs=2, name="qkps")
                            for h in range(2):
                                sl = bass.ts(h, 512)
                                for kt in range(CT):
                                    nc.tensor.matmul(
                                        qkps[:, sl],
                                        wqk_sb[:, tp, kt * P:(kt + 1) * P],
                                        h1[:, kt, sl],
                                        start=(kt == 0), stop=(kt == CT - 1))
                            qt = asb.tile([P, N], bf16, tag="qt", bufs=6,
                                          name="qt")
                            nc.vector.tensor_scalar(
                                out=qt[:], in0=qkps[:],
                                scalar1=bqk_sb[:, tp:tp + 1],
                                scalar2=IWS, op0=ALU.add, op1=ALU.mult)
                            # k: two zero-padded per-head tiles so scores use
                            # full-128 stationary (uniform PE config)
                            kps = aps.tile([P, N], f32, tag="sc", bufs=2,
                                           name="kps")
                            mt = CT + tp
                            for h in range(2):
                                sl = bass.ts(h, 512)
                                for kt in range(CT):
                                    nc.tensor.matmul(
                                        kps[:, sl],
                                        wqk_sb[:, mt, kt * P:(kt + 1) * P],
                                        h1[:, kt, sl],
                                        start=(kt == 0), stop=(kt == CT - 1))
                            ktA = asb.tile([P, N], bf16, tag="qt", bufs=6,
                                           name="ktA")
                            nc.gpsimd.memset(ktA[DH:P, :], 0.0)
                            nc.vector.tensor_scalar(
                                out=ktA[0:DH, :], in0=kps[0:DH, :],
                                scalar1=bqk_sb[0:DH, mt:mt + 1],
                                scalar2=IWS, op0=ALU.add, op1=ALU.mult)
                            ktB = asb.tile([P, N], bf16, tag="qt", bufs=6,
                                           name="ktB")
                            nc.gpsimd.memset(ktB[0:DH, :], 0.0)
                            nc.vector.tensor_scalar(
                                out=ktB[DH:P, :], in0=kps[DH:P, :],
                                scalar1=bqk_sb[DH:P, mt:mt + 1],
                                scalar2=IWS, op0=ALU.add, op1=ALU.mult)
                            return qt, ktA, ktB

                        def attn_jt(tp, jt, qt, ktA, ktB, av0, av1):
                            sc0 = aps.tile([P, N], f32, tag="sc", bufs=2,
                                           name="sc0")
                            sc1 = aps.tile([P, N], f32, tag="sc", bufs=2,
                                           name="sc1")
                            js = slice(jt * P, (jt + 1) * P)
                            for h in range(2):
                                sl = bass.ts(h, 512)
                                nc.tensor.matmul(
                                    sc0[:, sl], ktA[:, js], qt[:, sl])
                                nc.tensor.matmul(
                                    sc1[:, sl], ktB[:, js], qt[:, sl])
                            e0 = asb.tile([P, N], bf16, tag="exp", bufs=3,
                                          name="e0")
                            nc.scalar.activation(e0[:], sc0[:], AF.Exp,
                                                 scale=SCALE)
                            e1 = asb.tile([P, N], bf16, tag="exp", bufs=3,
                                          name="e1")
                            nc.scalar.activation(e1[:], sc1[:], AF.Exp,
                                                 scale=SCALE)
                            for h in range(2):
                                sl = bass.ts(h, 512)
                                nc.tensor.matmul(
                                    av0[:, sl], vsb[:, jt, 2 * tp, :],
                                    e0[:, sl],
                                    start=(jt == 0), stop=(jt == NT - 1))
                                nc.tensor.matmul(
                                    av1[:, sl], vsb[:, jt, 2 * tp + 1, :],
                                    e1[:, sl],
                                    start=(jt == 0), stop=(jt == NT - 1))

                        def finish_pair(tp, av0, av1):
                            ta = asb.tile([DH + 1, N], f32, tag="tst",
                                          bufs=2, name="ta")
                            nc.vector.tensor_copy(ta[DH:DH + 1, :],
                                                  av0[DH:DH + 1, :])
                            nc.gpsimd.dma_start(dtmp[2 * tp:2 * tp + 1, :],
                                                ta[DH:DH + 1, :])
                            tbd = asb.tile([DH + 1, N], f32, tag="tst",
                                           bufs=2, name="tbd")
                            nc.vector.tensor_copy(tbd[DH:DH + 1, :],
                                                  av1[DH:DH + 1, :])
                            nc.gpsimd.dma_start(dtmp[2 * tp + 1:2 * tp + 2, :],
                                                tbd[DH:DH + 1, :])
                            nc.vector.tensor_copy(osb[0:DH, tp, :],
                                                  av0[0:DH, :])
                            tb = asb.tile([DH + 1, N], bf16, tag="tsb",
                                          bufs=2, name="tb")
                            nc.vector.tensor_copy(tb[0:DH, :], av1[0:DH, :])
                            nc.gpsimd.dma_start(osb[DH:P, tp, :], tb[0:DH, :])

                        # ---- pair 0 with v-tiles interleaved ----
                        qt, ktA, ktB = qk_prod(0)
                        av0 = aps.tile([DH + 1, N], f32, tag="av", bufs=2,
                                       name="av0")
                        av1 = aps.tile([DH + 1, N], f32, tag="av", bufs=2,
                                       name="av1")
                        for jt in range(NT):
                            v_tile(jt)
                            attn_jt(0, jt, qt, ktA, ktB, av0, av1)
                        prev = (av0, av1)
                        for tp in range(1, CT):
                            qt, ktA, ktB = qk_prod(tp)
                            finish_pair(tp - 1, *prev)
                            nav0 = aps.tile([DH + 1, N], f32, tag="av",
                                            bufs=2, name="av0")
                            nav1 = aps.tile([DH + 1, N], f32, tag="av",
                                            bufs=2, name="av1")
                            for jt in range(NT):
                                attn_jt(tp, jt, qt, ktA, ktB, nav0, nav1)
                            prev = (nav0, nav1)
                        finish_pair(CT - 1, *prev)

                    # ---- denominators -> reciprocals -> f32r [2,6,N] ----
                    rf = attn.tile([12, N], f32, name="rf")
                    nc.vector.reciprocal_approx_fast(rf[:], dtmp[:])
                    rsb = attn.tile([12, N], f32r, name="rsb")
                    nc.vector.tensor_copy(rsb[:], rf[:])

                    # bias pre-add: xsb += bproj (LN1 consumers are done)
                    for mt in range(CT):
                        nc.scalar.activation(
                            xsb[:, mt, :], xsb[:, mt, :].bitcast(f32),
                            AF.Identity, bias=bproj_sb[:, mt:mt + 1],
                            scale=1.0)

                    prefetch(AF.Sqrt)
                    # ==== proj + residual (with softmax normalize) ====
                    with tc.tile_pool(name="pj_ps", bufs=1,
                                      space="PSUM") as pps:
                        accs = {}
                        for h in range(2):
                            sl = bass.ts(h, 512)
                            for kt in range(CT):
                                if h == 0:
                                    for h2 in range(2):
                                        sl2 = bass.ts(h2, 512)
                                        rt = pps.tile([P, 512], f32,
                                                      tag="rt", bufs=2,
                                                      name="rt")
                                        nc.tensor.matmul(
                                            rt[:],
                                            selS_sb[:, kt * P:(kt + 1) * P],
                                            rsb[:, sl2],
                                            start=True, stop=True)
                                        nc.vector.tensor_mul(
                                            osb[:, kt, sl2],
                                            osb[:, kt, sl2], rt[:])
                                for mt in range(CT):
                                    if kt == 0:
                                        accs[mt] = pps.tile(
                                            [P, 512], f32, tag="acc",
                                            bufs=6, name="acc")
                                    nc.tensor.matmul(
                                        accs[mt][:],
                                        wp_sb[:, kt, mt * P:(mt + 1) * P],
                                        osb[:, kt, sl],
                                        start=(kt == 0), stop=(kt == CT - 1))
                            for mt in range(CT):
                                nc.vector.scalar_tensor_tensor(
                                    out=xsb[:, mt, sl], in0=accs[mt][:],
                                    scalar=IWS, op0=ALU.mult,
                                    in1=xsb[:, mt, sl].bitcast(f32),
                                    op1=ALU.add)

                # ======== LN2 + MLP ========
                h2 = hpool.tile([P, CT, N], bf16, tag="h", name="h2")
                layer_norm_T(h2)
                for mt in range(CT):
                    nc.scalar.activation(
                        xsb[:, mt, :], xsb[:, mt, :].bitcast(f32),
                        AF.Identity, bias=bfc2_sb[:, mt:mt + 1], scale=1.0)
                with tc.tile_pool(name="mlp", bufs=1) as mw:
                    h3sb = mw.tile([P, HT, N], fp8, name="h3sb")
                    with tc.tile_pool(name="fc1_ps", bufs=1,
                                      space="PSUM") as f1p:
                        for ct in range(HT):
                            w1t = w1tiles[ct // 2]
                            h3ps = f1p.tile([P, N], f32, tag="h3ps", bufs=3,
                                            name="h3ps")
                            for h in range(2):
                                sl = bass.ts(h, 512)
                                for kt in range(CT):
                                    nc.tensor.matmul(
                                        h3ps[:, sl],
                                        w1t[:, ct % 2, kt * P:(kt + 1) * P],
                                        h2[:, kt, sl],
                                        start=(kt == 0), stop=(kt == CT - 1))
                            nc.scalar.activation(
                                h3sb[:, ct, :], h3ps[:], AF.Gelu,
                                bias=bfc1_sb[:, ct:ct + 1], scale=IWS)
                    with tc.tile_pool(name="fc2_ps", bufs=1,
                                      space="PSUM") as f2p:
                        for g in range(2):
                            f2ps = [f2p.tile([P, N], f32, tag=f"f2_{i}",
                                             bufs=1, name=f"f2ps{i}")
                                    for i in range(3)]
                            for c in range(HT // 2):
                                for i in range(3):
                                    for h in range(2):
                                        sl = bass.ts(h, 512)
                                        nc.tensor.matmul(
                                            f2ps[i][:, sl],
                                            w2tiles[c][:, :, g,
                                                       i * P:(i + 1) * P],
                                            h3sb[:, 2 * c:2 * c + 2, sl],
                                            start=(c == 0),
                                            stop=(c == HT // 2 - 1),
                                            perf_mode=
                                            mybir.MatmulPerfMode.DoubleRow)
                            for i in range(3):
                                mt = g * 3 + i
                                nc.vector.scalar_tensor_tensor(
                                    out=xsb[:, mt, :], in0=f2ps[i][:],
                                    scalar=IWS, op0=ALU.mult,
                                    in1=xsb[:, mt, :].bitcast(f32),
                                    op1=ALU.add)
                                if last:
                                    nc.gpsimd.dma_start(
                                        _t6(outT)[:, mt, :],
                                        xsb[:, mt, :].bitcast(f32))


_NC_CACHE = None


def _get_nc():
    global _NC_CACHE
    if _NC_CACHE is None:
        _NC_CACHE = build_nc()
    return _NC_CACHE


def _f8(a):
    return np.ascontiguousarray(a * WS).astype(ml_dtypes.float8_e4m3)


def _prep_shared(qkv_w, qkv_b, proj_w, proj_b, fc1_w, fc1_b, fc2_w, fc2_b,
                 ln1_g, ln1_b, ln2_g, ln2_b):
    c = lambda a: np.ascontiguousarray(np.asarray(a, dtype=np.float32))
    qkv_w = np.asarray(qkv_w, np.float32)
    proj_w = np.asarray(proj_w, np.float32)
    fc1_w = np.asarray(fc1_w, np.float32)
    g1 = np.asarray(ln1_g, np.float32)
    b1 = np.asarray(ln1_b, np.float32)
    g2 = np.asarray(ln2_g, np.float32)
    b2 = np.asarray(ln2_b, np.float32)
    qkv_wf = qkv_w * g1[:, None]
    qkv_bf = np.asarray(qkv_b, np.float32) + b1 @ qkv_w
    fc1_wf = fc1_w * g2[:, None]
    fc1_bf = np.asarray(fc1_b, np.float32) + b2 @ fc1_w
    consts = np.zeros((P, 48), np.float32)
    consts[:, 0:12] = (WS * qkv_bf[:2 * DIM]).reshape(12, P).T
    consts[:, 12:36] = fc1_bf.reshape(HT, P).T
    consts[:, 36:42] = c(proj_b).reshape(CT, P).T
    consts[:, 42:48] = c(fc2_b).reshape(CT, P).T
    selS = np.zeros((12, CT * P), np.float32)
    for kt in range(CT):
        selS[2 * kt, kt * P:kt * P + DH] = 1.0
        selS[2 * kt + 1, kt * P + DH:(kt + 1) * P] = 1.0
    return {
        "wqk": _f8(qkv_wf[:, :2 * DIM].reshape(CT, P, 12, P)
                   .transpose(2, 1, 0, 3).reshape(12, P, CT * P)),
        "wv": np.ascontiguousarray(
            qkv_wf[:, 2 * DIM:].reshape(CT, P, DIM).transpose(1, 0, 2)
        ).astype(ml_dtypes.bfloat16),
        "wproj": _f8(proj_w.reshape(CT, P, DIM).transpose(1, 0, 2)),
        "wfc1": _f8(fc1_wf.reshape(CT, P, HT, P).transpose(2, 1, 0, 3)
                    .reshape(HT, P, CT * P)),
        "wfc2": _f8(np.asarray(fc2_w, np.float32).reshape(HT, P, 2, 3 * P)
                    .transpose(0, 2, 1, 3)),
        "consts": consts,
        "vb": c(qkv_bf[2 * DIM:]),
        "selS": selS,
    }


def run(x, shared, **spmd_kwargs):
    nc = _get_nc()
    x = np.asarray(x, dtype=np.float32)
    in_maps = [
        {**shared, "xT": np.ascontiguousarray(x[b].T)} for b in range(B)
    ]
    res = run_bass_kernel_spmd(nc, in_maps, core_ids=list(range(B)),
                               **spmd_kwargs)
    out = np.stack([res.results[b]["outT"].T for b in range(B)])
    return out.astype(np.float32), res


def kernel(x, ln1_g, ln1_b, qkv_w, qkv_b, proj_w, proj_b,
           ln2_g, ln2_b, fc1_w, fc1_b, fc2_w, fc2_b):
    shared = _prep_shared(qkv_w, qkv_b, proj_w, proj_b, fc1_w, fc1_b,
                          fc2_w, fc2_b, ln1_g, ln1_b, ln2_g, ln2_b)
    out, _ = run(x, shared)
    return out

